# revision 12
# baseline (speedup 1.0000x reference)
"""MultiHeadAttention Trainium2 Bass kernel.

B=2, S=2048, D=768, H=12 (head dim 64). 8 NeuronCores:
core c -> batch b = c//4, head group g = c%4 (3 heads each).

Per-core dataflow (all matmul operands bf16 -> 1 cycle/row; PSUM fp32):
  - host supplies x.T [768, 2048] bf16; QKV projections contract over
    the partition dim with no on-device transpose
  - Q/K land in per-head [128, S] tiles padded with zero rows so every
    score / O-projection matmul contracts over K=128: K<128 matmuls run
    in a row-tiled PE mode that the HAM activity monitor ignores, which
    leaves the PE clock-gated at 1.2 GHz; zero-padding to full 128x128
    mode keeps the array at 2.4 GHz for the whole attention phase
  - V01/Q01/K01 project serially up front (V transposes inline); the
    64-wide V2/Q2/K2 chunks and the head-2 V transposes are injected as
    fillers into the first two ACT-bound attention passes
  - exp(ST + mask_bias) on the scalar engine (fp32 PSUM in, bf16 out)
  - PV matmul consumes the exp'd tile as the moving operand; a ones
    column prepended to V yields the softmax denominator in row 0 free
  - normalize: fp32 reciprocal (DVE) + partition broadcast (GpSimd) +
    DVE multiply -> bf16 oT rows 0..64 (row 0 = 1.0, nulled by the zero
    row 0 of wo; rows 65..127 zeroed for the K=128 O projection)
  - row-parallel O projection emits a partial [2048, 768] bf16; host
    sums the 4 head-group partials per batch and adds bo. O-projection
    column groups for the first half of the sequence are interleaved
    into the second attention pass to fill PE slack.
"""

import sys

if "/opt/trn_rl_repo" not in sys.path:
    sys.path.insert(0, "/opt/trn_rl_repo")

import numpy as np
import ml_dtypes

B, S, D, H = 2, 2048, 768, 12
NH = 64          # head dim
HPC = 3          # heads per core
JW = 192         # qkv columns per head group

# m-chunks of the fused qkv projection output (columns of wqkv):
# [V01 | Q01 | K01 | V2 | Q2 | K2]
MCH = [0, 128, 256, 384, 448, 512]
MSZ = [128, 128, 128, 64, 64, 64]

_CACHE = {}


def _build_nc():
    import concourse.bass as bass
    import concourse.tile as tile
    from concourse import bacc
    from concourse import mybir

    f32 = mybir.dt.float32
    f32r = mybir.dt.float32r
    bf16 = mybir.dt.bfloat16
    AF = mybir.ActivationFunctionType
    Alu = mybir.AluOpType

    nc = bacc.Bacc(None, target_bir_lowering=False, debug=False)

    xT_d = nc.dram_tensor("xT", [D, S], bf16, kind="ExternalInput")
    wqkv_d = nc.dram_tensor("wqkv", [D, 576], bf16, kind="ExternalInput")
    bias_d = nc.dram_tensor("biasp", [6, 128], f32, kind="ExternalInput")
    maskb_d = nc.dram_tensor("maskb", [16, 128], f32, kind="ExternalInput")
    # wo padded to 128 rows per head (row 0 and rows 65..127 zero)
    wo_d = nc.dram_tensor("wo3", [HPC, 128, D], bf16, kind="ExternalInput")
    ident_d = nc.dram_tensor("identd", [128, 192], bf16, kind="ExternalInput")
    out_d = nc.dram_tensor("out", [S, D], bf16, kind="ExternalOutput")

    with tile.TileContext(nc) as tc:
        with (
            tc.tile_pool(name="const", bufs=1) as constp,
            tc.tile_pool(name="xp", bufs=1) as xp,
            tc.tile_pool(name="qkv", bufs=1) as qkvp,
            tc.tile_pool(name="pp", bufs=3) as pp,
            tc.tile_pool(name="outp", bufs=2) as outp,
            tc.tile_pool(name="miscp", bufs=2) as miscp,
            tc.tile_pool(name="psA", bufs=2, space="PSUM") as psA,
            tc.tile_pool(name="psB", bufs=4, space="PSUM") as psB,
        ):
            # ---------------- constants ----------------
            wqkv_sb = constp.tile([128, 6, 576], bf16)
            nc.sync.dma_start(wqkv_sb, wqkv_d[:, :].rearrange("(o p) f -> p o f", p=128))
            bias_sb = constp.tile([128, 6], f32)
            nc.sync.dma_start(bias_sb, bias_d[:, :].rearrange("o p -> p o"))
            maskb_sb = constp.tile([128, 16], f32)
            nc.sync.dma_start(maskb_sb, maskb_d[:, :].rearrange("o p -> p o"))
            wo_sb = constp.tile([128, HPC, D], bf16)
            nc.sync.dma_start(wo_sb, wo_d[:, :, :].rearrange("h p f -> p h f"))
            identones = constp.tile([128, 192], bf16)
            nc.sync.dma_start(identones, ident_d[:, :])
            ident = identones[:, 0:128]
            ones_sb = identones[:, 128:192]

            # xT DMA'd in (kc, si) chunks so the first QKV matmuls can
            # start after ~1/4 of the input has landed
            xT_sb = xp.tile([128, 6, 4, 512], bf16)
            xT_r = xT_d[:, :].rearrange("(o p) f -> p o f", p=128)
            for si in range(4):
                for kc in range(6):
                    nc.sync.dma_start(
                        xT_sb[:, kc, si, :], xT_r[:, kc, si * 512 : (si + 1) * 512]
                    )

            # ---------------- padded Q/K/oT tiles ----------------
            qT0p = qkvp.tile([128, S], bf16)   # data rows 0..63
            kT0p = qkvp.tile([128, S], bf16)
            qT1p = qkvp.tile([128, S], bf16)   # data rows 64..127
            kT1p = qkvp.tile([128, S], bf16)
            qT2p = qkvp.tile([128, S], bf16)   # data rows 0..63
            kT2p = qkvp.tile([128, S], bf16)
            vT01 = qkvp.tile([128, S], bf16)
            vT2 = qkvp.tile([64, S], bf16)
            oTnps = [qkvp.tile([128, S], bf16, name=f"oTnp{h}") for h in range(HPC)]
            for t in (qT0p, kT0p, qT2p, kT2p):
                nc.gpsimd.memset(t[64:128, :], 0.0)
            for t in (qT1p, kT1p):
                nc.gpsimd.memset(t[0:64, :], 0.0)
            # rows 65..127 must be zero for the K=128 O projection; row
            # 64 is memset too (AP base must be 32-aligned) then
            # overwritten by the normalize write of rows 0..64. Row 0
            # ends up holding denom*recip = 1.0, nulled by wo's zero row.
            for t in oTnps:
                nc.gpsimd.memset(t[64:128, :], 0.0)

            # evac plan per m-chunk: list of (psum rows, target, target rows)
            evacs = [
                [(slice(0, 128), vT01, slice(0, 128))],
                [(slice(0, 64), qT0p, slice(0, 64)), (slice(64, 128), qT1p, slice(64, 128))],
                [(slice(0, 64), kT0p, slice(0, 64)), (slice(64, 128), kT1p, slice(64, 128))],
                [(slice(0, 64), vT2, slice(0, 64))],
                [(slice(0, 64), qT2p, slice(0, 64))],
                [(slice(0, 64), kT2p, slice(0, 64))],
            ]

            # v_sb[:, kc, 65h] = 1.0 (denominator lands at PV output row 0
            # so partition_broadcast can read it); cols 65h+1..65h+64 = V
            v_sb = qkvp.tile([128, 16, 3 * 65], bf16)
            v_sb_h = v_sb.rearrange("p k (h c) -> p k h c", c=65)
            for kc in range(16):
                nc.vector.tensor_copy(out=v_sb_h[:, kc, :, 0], in_=ones_sb[:, 0:3])

            def qkv_chunk(si, mi):
                mc, mst = MSZ[mi], MCH[mi]
                ps = psA.tile([128, 2, 512], f32, tag="st", name=f"ps{si}_{mi}")
                for kc in range(6):
                    nc.tensor.matmul(
                        ps[:mc, 0, :],
                        lhsT=wqkv_sb[:, kc, mst : mst + mc],
                        rhs=xT_sb[:, kc, si, :],
                        start=(kc == 0),
                        stop=(kc == 5),
                    )
                for prow, tgt, trow in evacs[mi]:
                    nc.vector.tensor_scalar(
                        out=tgt[trow, si * 512 : (si + 1) * 512],
                        in0=ps[prow, 0, :],
                        scalar1=bias_sb[prow, mi : mi + 1],
                        scalar2=None,
                        op0=Alu.add,
                    )

            def pt01(kc):
                # heads 0/1 V columns of v_sb for one 128-key chunk
                ks = slice(kc * 128, (kc + 1) * 128)
                pt = psB.tile([128, 512], bf16, tag="ot", name=f"pt{kc}")
                nc.tensor.transpose(pt[:, :128], vT01[:, ks], ident)
                nc.vector.tensor_copy(out=v_sb[:, kc, 1:65], in_=pt[:, 0:64])
                nc.vector.tensor_copy(out=v_sb[:, kc, 66:130], in_=pt[:, 64:128])

            def pt2f(kc):
                # head 2: on the "st" ring (fast consumers only) because the
                # "ot" ring holds live PV accumulators while fillers run
                ks = slice(kc * 128, (kc + 1) * 128)
                pt2 = psA.tile([128, 2, 512], bf16, tag="st", name=f"pt2_{kc}")
                nc.tensor.transpose(pt2[:, 0, :64], vT2[:, ks], ident[:64, :64])
                nc.vector.tensor_copy(out=v_sb[:, kc, 131:195], in_=pt2[:, 0, 0:64])

            def pt2batch(k0):
                for k in range(k0, k0 + 4):
                    pt2f(k)

            # ---------------- serial head: V01 / Q01 / K01 ----------------
            for si in range(4):
                for mi in range(3):
                    qkv_chunk(si, mi)
                for kc in range(4 * si, 4 * si + 4):
                    pt01(kc)

            # fillers consumed inside the qp0 h0/h1 attention passes
            fillers = []
            for mi in (3, 4):
                for si in range(4):
                    fillers.append((qkv_chunk, (si, mi)))
            for kc in range(0, 16, 4):
                fillers.append((pt2batch, (kc,)))
            for si in range(4):
                fillers.append((qkv_chunk, (si, 5)))

            # ---------------- attention ----------------
            heads = [(kT0p, qT0p, 0), (kT1p, qT1p, 65), (kT2p, qT2p, 130)]

            def oproj_group(si):
                po = psA.tile([128, 2, 512], f32, tag="st", name=f"po{si}")
                pof = po.rearrange("p a b -> p (a b)")
                for h in range(HPC):
                    lhsT = oTnps[h][:, si * 128 : (si + 1) * 128]
                    nc.tensor.matmul(
                        pof[:, 0:512],
                        lhsT=lhsT,
                        rhs=wo_sb[:, h, 0:512],
                        start=(h == 0),
                        stop=(h == 2),
                    )
                    nc.tensor.matmul(
                        pof[:, 512:768],
                        lhsT=lhsT,
                        rhs=wo_sb[:, h, 512:768],
                        start=(h == 0),
                        stop=(h == 2),
                    )
                ob = outp.tile([128, D], bf16, tag="ob", name=f"ob{si}")
                with nc.allow_low_precision(reason="bf16 output partials"):
                    nc.vector.tensor_copy(out=ob, in_=pof[:, :768])
                nc.sync.dma_start(out_d[si * 128 : (si + 1) * 128, :], ob)

            for qp in range(2):
                for h in range(HPC):
                    kt, qt, vc = heads[h]
                    oTn = oTnps[h]
                    ots = [
                        psB.tile([128, 512], f32, tag="ot", name=f"ot{h}_{qp}_{j}")
                        for j in range(2)
                    ]
                    for kc in range(16):
                        st = psA.tile([128, 2, 512], f32, tag="st")
                        for j in range(2):
                            qsl = slice(qp * 1024 + j * 512, qp * 1024 + (j + 1) * 512)
                            nc.tensor.matmul(
                                st[:, j, :],
                                lhsT=kt[:, kc * 128 : (kc + 1) * 128],
                                rhs=qt[:, qsl],
                                start=True,
                                stop=True,
                            )
                        p = pp.tile([128, 2, 512], bf16, tag="p")
                        nc.scalar.activation(
                            p, st, AF.Exp, bias=maskb_sb[:, kc : kc + 1], scale=1.0
                        )
                        for j in range(2):
                            nc.tensor.matmul(
                                ots[j][:65, :],
                                lhsT=v_sb[:, kc, vc : vc + 65],
                                rhs=p[:, j, :],
                                start=(kc == 0),
                                stop=(kc == 15),
                            )
                        # qp0 h0/h1: late QKV chunks; qp1: O-projection
                        # groups for the (already normalized) first half
                        if qp == 0 and h < 2 and kc % 2 == 1 and fillers:
                            fn, args = fillers.pop(0)
                            fn(*args)
                        if qp == 1 and kc in (4, 9, 13):
                            si = h * 3 + (kc == 9) + 2 * (kc == 13)
                            if si < 8:
                                oproj_group(si)
                    if qp == 0 and h == 1:
                        # anything not absorbed must precede the head-2 pass
                        for fn, args in fillers:
                            fn(*args)
                        fillers = []
                    for j in range(2):
                        ot = ots[j]
                        recip = miscp.tile([65, 512], f32r, tag="recip")
                        with nc.allow_low_precision(
                            reason="f32r is 4-byte; typed for the fp32r matmul verifier"
                        ):
                            nc.vector.reciprocal(
                                out=recip[0:1, :], in_=ot[0:1, :]
                            )
                        recb = miscp.tile([65, 512], f32r, tag="recb")
                        nc.gpsimd.partition_broadcast(recb[0:65, :], recip[0:1, :])
                        qsl = slice(qp * 1024 + j * 512, qp * 1024 + (j + 1) * 512)
                        with nc.allow_low_precision(
                            reason="bf16 oT feeds the bf16 O-projection matmul"
                        ):
                            nc.vector.tensor_tensor(
                                out=oTn[0:65, qsl],
                                in0=ot[0:65, :],
                                in1=recb[0:65, :],
                                op=Alu.mult,
                            )

            # ---------------- O projection tail ----------------
            for si in range(8, 16):
                oproj_group(si)

    nc.compile()
    return nc


def _get_nc():
    if "nc" not in _CACHE:
        _CACHE["nc"] = _build_nc()
    return _CACHE["nc"]


def make_in_maps(inputs):
    """Shard full inputs into the 8 per-core input maps."""
    bf16 = ml_dtypes.bfloat16
    x = np.asarray(inputs["inputs"], dtype=np.float32)
    masks = np.asarray(inputs["masks"])
    Wq = np.asarray(inputs["Wq"], dtype=np.float32)
    Wk = np.asarray(inputs["Wk"], dtype=np.float32)
    Wv = np.asarray(inputs["Wv"], dtype=np.float32)
    Wo = np.asarray(inputs["Wo"], dtype=np.float32)
    bq = np.asarray(inputs["bq"], dtype=np.float32)
    bk = np.asarray(inputs["bk"], dtype=np.float32)
    bv = np.asarray(inputs["bv"], dtype=np.float32)

    scale = np.float32(1.0 / np.sqrt(NH))
    in_maps = []
    for c in range(8):
        b, g = c // 4, c % 4
        j0 = JW * g
        xT = np.ascontiguousarray(x[b].T.astype(bf16))  # [768, 2048]
        wq = Wq[:, j0 : j0 + JW] * scale
        wk = Wk[:, j0 : j0 + JW]
        wv = Wv[:, j0 : j0 + JW]
        wqkv = np.ascontiguousarray(
            np.concatenate(
                [wv[:, 0:128], wq[:, 0:128], wk[:, 0:128],
                 wv[:, 128:192], wq[:, 128:192], wk[:, 128:192]],
                axis=1,
            ).astype(bf16)
        )
        bqg = bq[j0 : j0 + JW] * scale
        bkg = bk[j0 : j0 + JW]
        bvg = bv[j0 : j0 + JW]
        biasp = np.zeros((6, 128), np.float32)
        biasp[0, :] = bvg[0:128]
        biasp[1, :] = bqg[0:128]
        biasp[2, :] = bkg[0:128]
        biasp[3, :64] = bvg[128:192]
        biasp[4, :64] = bqg[128:192]
        biasp[5, :64] = bkg[128:192]
        maskb = np.where(masks[b] == 0, np.float32(-1e12), np.float32(0.0))
        maskb = np.ascontiguousarray(maskb.astype(np.float32).reshape(16, 128))
        wo3 = np.zeros((HPC, 128, D), np.float32)
        wo3[:, 1 : NH + 1, :] = Wo[j0 : j0 + JW, :].reshape(HPC, NH, D)
        wo3 = np.ascontiguousarray(wo3.astype(bf16))
        identd = np.zeros((128, 192), np.float32)
        identd[:, 0:128] = np.eye(128, dtype=np.float32)
        identd[:, 128:192] = 1.0
        identd = identd.astype(bf16)
        in_maps.append(
            {"xT": xT, "wqkv": wqkv, "biasp": biasp, "maskb": maskb,
             "wo3": wo3, "identd": identd}
        )
    return in_maps


def combine(results, inputs):
    bo = np.asarray(inputs["bo"], dtype=np.float32)
    out = np.zeros((B, S, D), np.float32)
    for c in range(8):
        out[c // 4] += results[c]["out"].astype(np.float32)
    out += bo
    return out


def kernel(**inputs):
    from concourse.bass_utils import run_bass_kernel_spmd

    nc = _get_nc()
    in_maps = make_in_maps(inputs)
    res = run_bass_kernel_spmd(nc, in_maps, list(range(8)))
    return combine(res.results, inputs)


# revision 13
# speedup vs baseline: 1.1312x; 1.1312x over previous
"""MultiHeadAttention Trainium2 Bass kernel.

B=2, S=2048, D=768, H=12 (head dim 64). 8 NeuronCores:
core c -> batch b = c//4, head group g = c%4 (3 heads each).

Per-core dataflow (all matmul operands bf16 -> 1 cycle/row; PSUM fp32):
  - host supplies x.T [768, 2048] bf16; QKV projections contract over
    the partition dim with no on-device transpose
  - Q/K land in per-head [128, S] tiles padded with zero rows so every
    score / O-projection matmul contracts over K=128: K<128 matmuls run
    in a row-tiled PE mode that the HAM activity monitor ignores, which
    leaves the PE clock-gated at 1.2 GHz; zero-padding to full 128x128
    mode keeps the array at 2.4 GHz for the whole attention phase
  - exp(ST + mask_bias) on the scalar engine (fp32 PSUM in, bf16 out)
  - PV matmul consumes the exp'd tile as the moving operand; a ones
    column appended to V yields the softmax denominator in row 64 free
  - normalize: fp32 reciprocal (DVE) + partition broadcast (GpSimd) +
    DVE multiply -> bf16 oT (rows 64..127 zeroed for the K=128 O proj)
  - row-parallel O projection emits a partial [2048, 768] fp32; host
    sums the 4 head-group partials per batch and adds bo. O-projection
    column groups for the first half of the sequence are interleaved
    into the second attention pass to fill PE slack under the
    ACT-bound softmax loop.
"""

import sys

if "/opt/trn_rl_repo" not in sys.path:
    sys.path.insert(0, "/opt/trn_rl_repo")

import numpy as np
import ml_dtypes

B, S, D, H = 2, 2048, 768, 12
NH = 64          # head dim
HPC = 3          # heads per core
JW = 192         # qkv columns per head group

# m-chunks of the fused qkv projection output (columns of wqkv):
# [Q01 | K01 | Q2 | K2 | V01 | V2]
MCH = [0, 128, 256, 320, 384, 512]
MSZ = [128, 128, 64, 64, 128, 64]

# whether the gpsimd partition_broadcast custom op is available; falls
# back to a K=1 PE broadcast matmul when False
USE_GPSIMD_BCAST = True

_CACHE = {}


def _build_nc():
    import concourse.bass as bass
    import concourse.tile as tile
    from concourse import bacc
    from concourse import mybir

    f32 = mybir.dt.float32
    f32r = mybir.dt.float32r
    bf16 = mybir.dt.bfloat16
    AF = mybir.ActivationFunctionType
    Alu = mybir.AluOpType

    nc = bacc.Bacc(None, target_bir_lowering=False, debug=False)

    xT_d = nc.dram_tensor("xT", [D, S], bf16, kind="ExternalInput")
    wqkv_d = nc.dram_tensor("wqkv", [D, 576], bf16, kind="ExternalInput")
    bias_d = nc.dram_tensor("biasp", [6, 128], f32, kind="ExternalInput")
    maskb_d = nc.dram_tensor("maskb", [16, 128], f32, kind="ExternalInput")
    # wo padded to 128 rows per head (rows 64..127 zero)
    wo_d = nc.dram_tensor("wo3", [HPC, 128, D], bf16, kind="ExternalInput")
    ident_d = nc.dram_tensor("identd", [128, 192], bf16, kind="ExternalInput")
    onesf_d = nc.dram_tensor("onesf", [128, 128], f32r, kind="ExternalInput")
    out_d = nc.dram_tensor("out", [S, D], bf16, kind="ExternalOutput")

    with tile.TileContext(nc) as tc:
        with (
            tc.tile_pool(name="const", bufs=1) as constp,
            tc.tile_pool(name="xp", bufs=1) as xp,
            tc.tile_pool(name="qkv", bufs=1) as qkvp,
            tc.tile_pool(name="pp", bufs=3) as pp,
            tc.tile_pool(name="outp", bufs=2) as outp,
            tc.tile_pool(name="miscp", bufs=2) as miscp,
            tc.tile_pool(name="psA", bufs=2, space="PSUM") as psA,
            tc.tile_pool(name="psB", bufs=4, space="PSUM") as psB,
        ):
            # ---------------- constants ----------------
            wqkv_sb = constp.tile([128, 6, 576], bf16)
            nc.sync.dma_start(wqkv_sb, wqkv_d[:, :].rearrange("(o p) f -> p o f", p=128))
            bias_sb = constp.tile([128, 6], f32)
            nc.sync.dma_start(bias_sb, bias_d[:, :].rearrange("o p -> p o"))
            maskb_sb = constp.tile([128, 16], f32)
            nc.sync.dma_start(maskb_sb, maskb_d[:, :].rearrange("o p -> p o"))
            wo_sb = constp.tile([128, HPC, D], bf16)
            nc.sync.dma_start(wo_sb, wo_d[:, :, :].rearrange("h p f -> p h f"))
            identones = constp.tile([128, 192], bf16)
            nc.sync.dma_start(identones, ident_d[:, :])
            ident = identones[:, 0:128]
            ones_sb = identones[:, 128:192]
            onesf_sb = constp.tile([128, 128], f32r)
            nc.sync.dma_start(onesf_sb, onesf_d[:, :])

            # xT DMA'd in (kc, si) chunks so the first QKV matmuls can
            # start after ~1/4 of the input has landed
            xT_sb = xp.tile([128, 6, 4, 512], bf16)
            xT_r = xT_d[:, :].rearrange("(o p) f -> p o f", p=128)
            for si in range(4):
                for kc in range(6):
                    nc.sync.dma_start(
                        xT_sb[:, kc, si, :], xT_r[:, kc, si * 512 : (si + 1) * 512]
                    )

            # ---------------- padded Q/K/oT tiles ----------------
            qT0p = qkvp.tile([128, S], bf16)   # data rows 0..63
            kT0p = qkvp.tile([128, S], bf16)
            qT1p = qkvp.tile([128, S], bf16)   # data rows 64..127
            kT1p = qkvp.tile([128, S], bf16)
            qT2p = qkvp.tile([128, S], bf16)   # data rows 0..63
            kT2p = qkvp.tile([128, S], bf16)
            vT01 = qkvp.tile([128, S], bf16)
            vT2 = qkvp.tile([64, S], bf16)
            oTnps = [qkvp.tile([128, S], bf16, name=f"oTnp{h}") for h in range(HPC)]
            # zero the padding rows once (idle engine; disjoint from the
            # data-row writes so it overlaps the projections)
            for t in (qT0p, kT0p, qT2p, kT2p):
                nc.gpsimd.memset(t[64:128, :], 0.0)
            for t in (qT1p, kT1p):
                nc.gpsimd.memset(t[0:64, :], 0.0)
            # rows 65..127 must be zero for the K=128 O projection; row
            # 64 is memset too (base must be 32-aligned) then overwritten
            # by the normalize write of rows 0..64. Row 0 ends up holding
            # denom*recip = 1.0, nulled by the zero row 0 of wo.
            for t in oTnps:
                nc.gpsimd.memset(t[64:128, :], 0.0)

            # evac plan per m-chunk: list of (psum rows, target, target rows)
            evacs = [
                [(slice(0, 64), qT0p, slice(0, 64)), (slice(64, 128), qT1p, slice(64, 128))],
                [(slice(0, 64), kT0p, slice(0, 64)), (slice(64, 128), kT1p, slice(64, 128))],
                [(slice(0, 64), qT2p, slice(0, 64))],
                [(slice(0, 64), kT2p, slice(0, 64))],
                [(slice(0, 128), vT01, slice(0, 128))],
                [(slice(0, 64), vT2, slice(0, 64))],
            ]

            # ---------------- QKV projections ----------------
            for si in range(4):
                for mi in range(6):
                    mc, mst = MSZ[mi], MCH[mi]
                    ps = psA.tile([128, 2, 512], f32, tag="st", name=f"ps{si}_{mi}")
                    for kc in range(6):
                        nc.tensor.matmul(
                            ps[:mc, 0, :],
                            lhsT=wqkv_sb[:, kc, mst : mst + mc],
                            rhs=xT_sb[:, kc, si, :],
                            start=(kc == 0),
                            stop=(kc == 5),
                        )
                    for prow, tgt, trow in evacs[mi]:
                        nc.vector.tensor_scalar(
                            out=tgt[trow, si * 512 : (si + 1) * 512],
                            in0=ps[prow, 0, :],
                            scalar1=bias_sb[prow, mi : mi + 1],
                            scalar2=None,
                            op0=Alu.add,
                        )

            # ---------------- V natural layout (+ ones col) ----------------
            # v_sb[:, kc, 65h] = 1.0 (denominator row lands at PV output
            # row 0 so partition_broadcast can read it); cols 65h+1..65h+64
            # = V head h rows kc*128..
            v_sb = qkvp.tile([128, 16, 3 * 65], bf16)
            v_sb_h = v_sb.rearrange("p k (h c) -> p k h c", c=65)
            for kc in range(16):
                nc.vector.tensor_copy(
                    out=v_sb_h[:, kc, :, 0], in_=ones_sb[:, 0:3]
                )
                ks = slice(kc * 128, (kc + 1) * 128)
                pt = psB.tile([128, 512], bf16, tag="ot", name=f"pt{kc}")
                nc.tensor.transpose(pt[:, :128], vT01[:, ks], ident)
                nc.vector.tensor_copy(out=v_sb[:, kc, 1:65], in_=pt[:, 0:64])
                nc.vector.tensor_copy(out=v_sb[:, kc, 66:130], in_=pt[:, 64:128])
                pt2 = psB.tile([128, 512], bf16, tag="ot", name=f"pt2_{kc}")
                nc.tensor.transpose(pt2[:, :64], vT2[:, ks], ident[:64, :64])
                nc.vector.tensor_copy(out=v_sb[:, kc, 131:195], in_=pt2[:, 0:64])

            # ---------------- attention ----------------
            heads = [(kT0p, qT0p, 0), (kT1p, qT1p, 65), (kT2p, qT2p, 130)]

            def oproj_group(si):
                po = psA.tile([128, 2, 512], f32, tag="st", name=f"po{si}")
                pof = po.rearrange("p a b -> p (a b)")
                for h in range(HPC):
                    lhsT = oTnps[h][:, si * 128 : (si + 1) * 128]
                    nc.tensor.matmul(
                        pof[:, 0:512],
                        lhsT=lhsT,
                        rhs=wo_sb[:, h, 0:512],
                        start=(h == 0),
                        stop=(h == 2),
                    )
                    nc.tensor.matmul(
                        pof[:, 512:768],
                        lhsT=lhsT,
                        rhs=wo_sb[:, h, 512:768],
                        start=(h == 0),
                        stop=(h == 2),
                    )
                ob = outp.tile([128, D], bf16, tag="ob", name=f"ob{si}")
                with nc.allow_low_precision(reason="bf16 output partials"):
                    nc.vector.tensor_copy(out=ob, in_=pof[:, :768])
                nc.sync.dma_start(out_d[si * 128 : (si + 1) * 128, :], ob)

            for qp in range(2):
                for h in range(HPC):
                    kt, qt, vc = heads[h]
                    oTn = oTnps[h]
                    ots = [
                        psB.tile([128, 512], f32, tag="ot", name=f"ot{h}_{qp}_{j}")
                        for j in range(2)
                    ]
                    for kc in range(16):
                        st = psA.tile([128, 2, 512], f32, tag="st")
                        for j in range(2):
                            qsl = slice(qp * 1024 + j * 512, qp * 1024 + (j + 1) * 512)
                            nc.tensor.matmul(
                                st[:, j, :],
                                lhsT=kt[:, kc * 128 : (kc + 1) * 128],
                                rhs=qt[:, qsl],
                                start=True,
                                stop=True,
                            )
                        p = pp.tile([128, 2, 512], bf16, tag="p")
                        nc.scalar.activation(
                            p, st, AF.Exp, bias=maskb_sb[:, kc : kc + 1], scale=1.0
                        )
                        for j in range(2):
                            nc.tensor.matmul(
                                ots[j][:65, :],
                                lhsT=v_sb[:, kc, vc : vc + 65],
                                rhs=p[:, j, :],
                                start=(kc == 0),
                                stop=(kc == 15),
                            )
                        # fill PE slack under the ACT-bound softmax with
                        # O-projection groups for the first qp half (si 0..7
                        # only: those columns were normalized during qp 0)
                        if qp == 1 and kc in (4, 9, 13):
                            si = h * 3 + (kc == 9) + 2 * (kc == 13)
                            if si < 8:
                                oproj_group(si)
                    for j in range(2):
                        ot = ots[j]
                        recip = miscp.tile([65, 512], f32r, tag="recip")
                        with nc.allow_low_precision(
                            reason="f32r is 4-byte; typed for the fp32r matmul verifier"
                        ):
                            nc.vector.reciprocal(
                                out=recip[0:1, :], in_=ot[0:1, :]
                            )
                        recb = miscp.tile([65, 512], f32r, tag="recb")
                        if USE_GPSIMD_BCAST:
                            nc.gpsimd.partition_broadcast(
                                recb[0:65, :], recip[0:1, :]
                            )
                        else:
                            rbp = psA.tile([128, 2, 512], f32, tag="st", name=f"rbp{h}{qp}{j}")
                            nc.tensor.matmul(
                                rbp[:65, 0, :],
                                lhsT=onesf_sb[0:1, 0:65],
                                rhs=recip[0:1, :],
                                start=True,
                                stop=True,
                            )
                            nc.vector.tensor_copy(out=recb, in_=rbp[:65, 0, :])
                        qsl = slice(qp * 1024 + j * 512, qp * 1024 + (j + 1) * 512)
                        with nc.allow_low_precision(
                            reason="bf16 oT feeds the bf16 O-projection matmul"
                        ):
                            nc.vector.tensor_tensor(
                                out=oTn[0:65, qsl],
                                in0=ot[0:65, :],
                                in1=recb[0:65, :],
                                op=Alu.mult,
                            )

            # ---------------- O projection tail ----------------
            for si in range(8, 16):
                oproj_group(si)

    nc.compile()
    return nc


def _get_nc():
    if "nc" not in _CACHE:
        _CACHE["nc"] = _build_nc()
    return _CACHE["nc"]


def make_in_maps(inputs):
    """Shard full inputs into the 8 per-core input maps."""
    bf16 = ml_dtypes.bfloat16
    x = np.asarray(inputs["inputs"], dtype=np.float32)
    masks = np.asarray(inputs["masks"])
    Wq = np.asarray(inputs["Wq"], dtype=np.float32)
    Wk = np.asarray(inputs["Wk"], dtype=np.float32)
    Wv = np.asarray(inputs["Wv"], dtype=np.float32)
    Wo = np.asarray(inputs["Wo"], dtype=np.float32)
    bq = np.asarray(inputs["bq"], dtype=np.float32)
    bk = np.asarray(inputs["bk"], dtype=np.float32)
    bv = np.asarray(inputs["bv"], dtype=np.float32)

    scale = np.float32(1.0 / np.sqrt(NH))
    in_maps = []
    for c in range(8):
        b, g = c // 4, c % 4
        j0 = JW * g
        xT = np.ascontiguousarray(x[b].T.astype(bf16))  # [768, 2048]
        wq = Wq[:, j0 : j0 + JW] * scale
        wk = Wk[:, j0 : j0 + JW]
        wv = Wv[:, j0 : j0 + JW]
        wqkv = np.ascontiguousarray(
            np.concatenate(
                [wq[:, 0:128], wk[:, 0:128], wq[:, 128:192], wk[:, 128:192],
                 wv[:, 0:128], wv[:, 128:192]],
                axis=1,
            ).astype(bf16)
        )
        bqg = bq[j0 : j0 + JW] * scale
        bkg = bk[j0 : j0 + JW]
        bvg = bv[j0 : j0 + JW]
        biasp = np.zeros((6, 128), np.float32)
        biasp[0, :] = bqg[0:128]
        biasp[1, :] = bkg[0:128]
        biasp[2, :64] = bqg[128:192]
        biasp[3, :64] = bkg[128:192]
        biasp[4, :] = bvg[0:128]
        biasp[5, :64] = bvg[128:192]
        maskb = np.where(masks[b] == 0, np.float32(-1e12), np.float32(0.0))
        maskb = np.ascontiguousarray(maskb.astype(np.float32).reshape(16, 128))
        wo3 = np.zeros((HPC, 128, D), np.float32)
        wo3[:, 1 : NH + 1, :] = Wo[j0 : j0 + JW, :].reshape(HPC, NH, D)
        wo3 = np.ascontiguousarray(wo3.astype(bf16))
        identd = np.zeros((128, 192), np.float32)
        identd[:, 0:128] = np.eye(128, dtype=np.float32)
        identd[:, 128:192] = 1.0
        identd = identd.astype(bf16)
        onesf = np.ones((128, 128), np.float32)
        in_maps.append(
            {"xT": xT, "wqkv": wqkv, "biasp": biasp, "maskb": maskb,
             "wo3": wo3, "identd": identd, "onesf": onesf}
        )
    return in_maps


def combine(results, inputs):
    bo = np.asarray(inputs["bo"], dtype=np.float32)
    out = np.zeros((B, S, D), np.float32)
    for c in range(8):
        out[c // 4] += results[c]["out"].astype(np.float32)
    out += bo
    return out


def kernel(**inputs):
    from concourse.bass_utils import run_bass_kernel_spmd

    nc = _get_nc()
    in_maps = make_in_maps(inputs)
    res = run_bass_kernel_spmd(nc, in_maps, list(range(8)))
    return combine(res.results, inputs)
